# revision 1
# baseline (speedup 1.0000x reference)
import numpy as np
from concurrent.futures import ThreadPoolExecutor

# MoE gate routing for nn_Gate_50062138802428.
# x: [32768, 2048] f32, weight: [64, 2048] f32.
# Returns (weights [32768,6] f32, indices [32768,6] i32) matching
# softmax(x @ W^T) -> top-6 (values sorted desc, ties -> lowest index).

TOPK = 6
N_SHARDS = 8  # token-dim data parallel, per sharding hint


def _route_shard(x_shard: np.ndarray, weight_t: np.ndarray):
    logits = x_shard @ weight_t  # [n, 64] f32
    m = logits.max(axis=-1, keepdims=True)
    e = np.exp(logits - m, dtype=np.float32)
    scores = e / e.sum(axis=-1, keepdims=True, dtype=np.float32)

    # top-6 of 64: argpartition then stable sort desc (lowest index on ties)
    part = np.argpartition(-scores, TOPK - 1, axis=-1)[:, :TOPK]
    pvals = np.take_along_axis(scores, part, axis=-1)
    order = np.argsort(-pvals, axis=-1, kind="stable")
    idx_unordered = np.take_along_axis(part, order, axis=-1)
    vals = np.take_along_axis(pvals, order, axis=-1)
    # stable tie-break by expert index among equal values
    for j in range(TOPK - 1):
        tie = vals[:, j] == vals[:, j + 1]
        if tie.any():
            swap = tie & (idx_unordered[:, j] > idx_unordered[:, j + 1])
            if swap.any():
                a = idx_unordered[swap, j].copy()
                idx_unordered[swap, j] = idx_unordered[swap, j + 1]
                idx_unordered[swap, j + 1] = a
    return vals.astype(np.float32), idx_unordered.astype(np.int32)


def kernel(x: np.ndarray, weight: np.ndarray):
    x = np.ascontiguousarray(x, dtype=np.float32)
    weight_t = np.ascontiguousarray(weight.astype(np.float32).T)  # [2048, 64]
    n = x.shape[0]
    bounds = [(i * n // N_SHARDS, (i + 1) * n // N_SHARDS) for i in range(N_SHARDS)]
    with ThreadPoolExecutor(max_workers=N_SHARDS) as ex:
        parts = list(ex.map(lambda b: _route_shard(x[b[0]:b[1]], weight_t), bounds))
    weights_out = np.concatenate([p[0] for p in parts], axis=0)
    indices_out = np.concatenate([p[1] for p in parts], axis=0)
    return weights_out, indices_out



# revision 2
# speedup vs baseline: 1875.1540x; 1875.1540x over previous
"""MoE gate routing (nn_Gate) on 8 Trainium2 NeuronCores via Bass/Tile.

x: [32768, 2048] f32, weight: [64, 2048] f32.
Returns (weights [32768, 6] f32, indices [32768, 6] i32) matching
softmax(x @ W^T) -> top-6 (values sorted descending).

Sharding: x split along the token dim across 8 cores (data parallel);
the tiny gate weight is replicated.  Per core:
  - DMA x tiles [128, 2048] f32 natural layout (contiguous, full-rate)
  - PE transpose-mode 128x128 fp32 blocks -> PSUM (bit-exact)
  - DVE/ACT copy PSUM->SBUF with cast to bf16 (chunk-major xT staging)
  - W-stationary bf16 matmuls, [64, 512] f32 PSUM accumulation over
    16 dim-chunks (logits^T)
  - small PE transpose back to token-major, ACT exp (+accumulated sum),
    DVE max8/max_index8 top-k, scale by reciprocal of the softmax sum
"""

import os
import numpy as np
import ml_dtypes

N_CORES = 8
N_FULL, DIM, E = 32768, 2048, 64
NTOK = N_FULL // N_CORES  # tokens per core
P = 128
KCH = DIM // P            # dim chunks of 128
GROUP = 512               # tokens per matmul group (one PSUM bank)
NGROUPS = NTOK // GROUP
TPG = GROUP // P          # 128-token tiles per group
TOPK = 6

_cache = {}


def _build():
    if "nc" in _cache:
        return _cache["nc"]

    import concourse.mybir as mybir
    import concourse.tile as tile
    from concourse import bacc
    from concourse.masks import make_identity

    f32 = mybir.dt.float32
    bf16 = mybir.dt.bfloat16

    nc = bacc.Bacc(
        "TRN2",
        target_bir_lowering=False,
        debug=False,
        enable_asserts=False,
        num_devices=N_CORES,
    )
    x_d = nc.dram_tensor("x", [NTOK, DIM], f32, kind="ExternalInput").ap()
    wt_d = nc.dram_tensor("wt", [KCH, P, E], bf16, kind="ExternalInput").ap()
    ow_d = nc.dram_tensor("ow", [NTOK, 8], f32, kind="ExternalOutput").ap()
    oi_d = nc.dram_tensor("oi", [NTOK, 8], mybir.dt.uint32, kind="ExternalOutput").ap()

    with tile.TileContext(nc) as tc:
        with (
            tc.tile_pool(name="const", bufs=1) as cpool,
            tc.tile_pool(name="xa", bufs=8) as xa_pool,
            tc.tile_pool(name="xt", bufs=2) as xt_pool,
            tc.tile_pool(name="small", bufs=2) as sm_pool,
            tc.tile_pool(name="tp_ps", bufs=3, space="PSUM") as tp_pool,
            tc.tile_pool(name="acc_ps", bufs=2, space="PSUM") as acc_pool,
            tc.tile_pool(name="lt_ps", bufs=2, space="PSUM") as ltp_pool,
        ):
            ident = cpool.tile([P, P], f32)
            make_identity(nc, ident)
            wt_sb = cpool.tile([P, KCH, E], bf16)
            nc.scalar.dma_start(wt_sb, wt_d.rearrange("k p e -> p k e"))

            for g in range(NGROUPS):
                xa = []
                for t in range(TPG):
                    xa_t = xa_pool.tile([P, DIM], f32, tag="xa")
                    row = (g * TPG + t) * P
                    nc.sync.dma_start(xa_t, x_d[row : row + P, :])
                    xa.append(xa_t)

                # transpose x tiles into chunk-major bf16 staging [P, KCH, GROUP]
                xt = xt_pool.tile([P, KCH, GROUP], bf16, tag="xt")
                ci = g  # alternate copy engine per unit for DVE/ACT balance
                for t in range(TPG):
                    for r in range(KCH // 4):
                        tp = tp_pool.tile([P, 4, P], f32, tag="tp")
                        for dk in range(4):
                            k = 4 * r + dk
                            nc.tensor.transpose(
                                tp[:, dk, :], xa[t][:, k * P : (k + 1) * P], ident
                            )
                        dst = xt[:, 4 * r : 4 * r + 4, t * P : (t + 1) * P]
                        if ci % 2 == 0:
                            nc.vector.tensor_copy(out=dst, in_=tp)
                        else:
                            nc.scalar.copy(out=dst, in_=tp)
                        ci += 1

                # logits^T [E, GROUP] accumulated over dim chunks
                acc = acc_pool.tile([E, GROUP], f32, tag="acc")
                for k in range(KCH):
                    nc.tensor.matmul(
                        acc,
                        wt_sb[:, k, :],
                        xt[:, k, :],
                        start=(k == 0),
                        stop=(k == KCH - 1),
                    )

                # back to token-major [P, TPG, E]
                ltT = sm_pool.tile([E, GROUP], f32, tag="ltT")
                nc.scalar.copy(out=ltT, in_=acc)
                ltp = ltp_pool.tile([P, TPG, E], f32, tag="ltp")
                for t in range(TPG):
                    nc.tensor.transpose(
                        ltp[:, t, :], ltT[:, t * P : (t + 1) * P], ident[:E, :E]
                    )
                lt2 = sm_pool.tile([P, TPG, E], f32, tag="lt2")
                nc.vector.tensor_copy(out=lt2, in_=ltp)

                # softmax + top-8 (top-6 taken on host)
                e_sb = sm_pool.tile([P, TPG, E], f32, tag="esb")
                s4 = sm_pool.tile([P, TPG], f32, tag="s4")
                r4 = sm_pool.tile([P, TPG], f32, tag="r4")
                v8 = sm_pool.tile([P, TPG, 8], f32, tag="v8")
                i8 = sm_pool.tile([P, TPG, 8], mybir.dt.uint32, tag="i8")
                w8 = sm_pool.tile([P, TPG, 8], f32, tag="w8")
                for t in range(TPG):
                    nc.scalar.activation(
                        e_sb[:, t, :],
                        lt2[:, t, :],
                        mybir.ActivationFunctionType.Exp,
                        accum_out=s4[:, t : t + 1],
                    )
                nc.vector.reciprocal(r4, s4)
                for t in range(TPG):
                    nc.vector.max(out=v8[:, t, :], in_=e_sb[:, t, :])
                    nc.vector.max_index(
                        out=i8[:, t, :], in_max=v8[:, t, :], in_values=e_sb[:, t, :]
                    )
                nc.vector.tensor_tensor(
                    w8,
                    v8,
                    r4[:, :, None].to_broadcast([P, TPG, 8]),
                    mybir.AluOpType.mult,
                )

                rows = ow_d[g * GROUP : (g + 1) * GROUP, :]
                nc.scalar.dma_start(rows.rearrange("(t p) j -> p t j", p=P), w8)
                irows = oi_d[g * GROUP : (g + 1) * GROUP, :]
                nc.scalar.dma_start(irows.rearrange("(t p) j -> p t j", p=P), i8)

    nc.compile()
    _cache["nc"] = nc
    return nc


last_results = None  # BassKernelResults of the most recent run (for test harness)


def kernel(x, weight):
    global last_results
    nc = _build()
    from concourse import bass_utils

    x = np.ascontiguousarray(np.asarray(x), dtype=np.float32)
    w = np.asarray(weight, dtype=np.float32)
    wt = np.ascontiguousarray(w.T).reshape(KCH, P, E).astype(ml_dtypes.bfloat16)

    in_maps = [
        {"x": x[i * NTOK : (i + 1) * NTOK], "wt": wt} for i in range(N_CORES)
    ]
    res = bass_utils.run_bass_kernel_spmd(nc, in_maps, core_ids=list(range(N_CORES)))
    last_results = res

    w_out = np.concatenate([r["ow"][:, :TOPK] for r in res.results], axis=0)
    i_out = np.concatenate(
        [r["oi"][:, :TOPK].astype(np.int32) for r in res.results], axis=0
    )
    return np.ascontiguousarray(w_out), np.ascontiguousarray(i_out)
